# revision 1
# baseline (speedup 1.0000x reference)
"""Biaffine (trilinear + concat-linear) kernel for Trainium2, 8-core SPMD.

logits[b,x,y,o] = sum_ij in1[b,x,i] * w1[i,o,j] * in2[b,y,j]
               + termA[b,x,o] + termB[b,y,o] + bias[o]
  termA[b,x,o] = sum_i in1[b,x,i] * w2[i,o]
  termB[b,y,o] = sum_j in1[b,y,j] * w2[IN+j,o]   (both halves from input1!)
  bias[o]      = w2[2*IN,o]

Sharding: core c handles batch b=c//2, x-range [x0, x0+256), x0=256*(c%2).
w1/w2 replicated. Per core, two chained matmul phases over o-chunks of OC
(w1 is streamed through SBUF exactly once per core, batched OB o's per DMA,
host pre-casts it to bf16 to halve HBM traffic):
  phase 1: temp[j, o, x] = sum_i w1[i,o,j] * in1[x,i]
           (stationary = w1 128x128 tile, moving = in1^T [128, 256], fp32
           PSUM accumulation over 4 i-blocks, result stored bf16)
  phase 2: out[x, y] (per o) = sum_jblk temp-slice[j, x] @ in2T[j, y]
           + selector-matmul: lhsT[k,m] = identw[k,o] (free-broadcast AP)
             x rhs TBb[k,y]  ==> adds termB[y,o]+bias[o] to every x row
           then PSUM->SBUF drain + termA[x,o] bias-add on the ACT engine
           (scalar.activation Identity with per-partition bias) -- keeping
           DVE free for the phase-1 copies; DVE per-op DRAIN overhead made
           DVE the critical path when it carried both streams
temp is double-buffered so phase 1 of chunk N+1 overlaps phase 2 of chunk N.
Device output layout [x, o, y] so every output DMA line is >=14KB
contiguous; the host transposes to [x, y, o] while unsharding.

The selector matmul MUST use the bf16 identity (identw): with an fp32
zero-stride broadcast stationary the weight load takes a pathological slow
path and the whole main loop nearly doubles (measured ~0.96 ms -> ~0.6 ms
per core after switching it to bf16). Moving the phase-2 PSUM drain from
DVE to ACT measured ~2.4-2.8 ms faster over 6 chained iterations in a
same-window interleaved R6-vs-R6 A/B (bench3.py) — the most reliable
differential this tunnel allows. Absolute main-loop estimates across
measurement windows span ~0.5-0.75 ms/core (window-dependent bias of the
bimodal ~37/78 ms tunnel round-trip); cost-model TimelineSim says 0.47 ms.
jb_pack=True (single DVE drain for two packed PSUM groups) measured
neutral-to-worse; left off.
"""

import numpy as np

B, S, IN, OUT = 4, 512, 512, 112
N_CORES = 8
P = 128


def split_sync_waits(nc, max_waits=1):
    """The walrus codegen in this toolchain rejects instructions carrying
    more than a few semaphore waits ("Too many sync wait commands").
    Hoist overflow waits onto NoOps inserted just before the instruction,
    on the same engine (semantically identical: the sequencer blocks on
    each wait in order)."""
    import concourse.mybir as mybir

    n_split = 0
    for f in nc.m.functions:
        for bb in f.blocks:
            new_insts = []
            for inst in bb.instructions:
                si = inst.sync_info
                if si is not None and si.on_wait and len(si.on_wait) > max_waits:
                    waits = list(si.on_wait)
                    overflow, keep = waits[:-max_waits], waits[-max_waits:]
                    for k in range(0, len(overflow), max_waits):
                        chunk = overflow[k:k + max_waits]
                        nop = mybir.InstNoOp(
                            name=f"{inst.name}_wsplit{k}",
                            opcode="NoOp",
                            engine=inst.engine,
                            sync_info=mybir.SyncInfo(on_wait=chunk, on_update=[]),
                        )
                        new_insts.append(nop)
                        n_split += 1
                    si.on_wait = keep
                new_insts.append(inst)
            bb.instructions[:] = new_insts
    return n_split


def build_nc(S_=S, IN_=IN, OUT_=OUT, XW=256, OC=14, OG=7, OB=7, w1_bf16=True,
             temp_bufs=2, split_waits=True, repeat=1, only_phase=0,
             act_drain=True, jb_pack=False):
    """Build the per-core Bass module. All 8 cores run the same program on
    their own input slices (SPMD)."""
    import concourse.bass as bass
    import concourse.mybir as mybir
    import concourse.tile as tile
    from concourse.masks import make_identity

    f32 = mybir.dt.float32
    wdt = mybir.dt.bfloat16 if w1_bf16 else f32

    KI = IN_ // P          # number of 128-blocks of the i/j contraction dims
    YB = S_ // P           # y 128-blocks
    XB = XW // P           # x 128-blocks per core
    NCH = OUT_ // OC       # o-chunks
    assert OC % OG == 0 and OC % OB == 0

    nc = bass.Bass()
    in1x = nc.dram_tensor("in1x", [XW, IN_], f32, kind="ExternalInput")
    in1f = nc.dram_tensor("in1f", [S_, IN_], f32, kind="ExternalInput")
    in2f = nc.dram_tensor("in2f", [S_, IN_], f32, kind="ExternalInput")
    w1 = nc.dram_tensor("w1", [IN_, OUT_, IN_], wdt, kind="ExternalInput")
    w2 = nc.dram_tensor("w2", [2 * IN_ + 1, OUT_], f32, kind="ExternalInput")
    outp = nc.dram_tensor("outp", [XW, OUT_, S_], f32, kind="ExternalOutput")

    with tile.TileContext(nc) as tc:
        with tc.tile_pool(name="persist", bufs=1) as pers:
            # persistent SBUF tensors
            in1Tx = pers.tile([P, KI, XW], f32, name="in1Tx")   # in1x^T
            in1Tf = pers.tile([P, KI, S_], f32, name="in1Tf")   # in1f^T
            in2T = pers.tile([P, KI, S_], wdt, name="in2T")     # in2f^T
            wA = pers.tile([P, KI, OUT_], f32, name="wA")
            wB = pers.tile([P, KI, OUT_], f32, name="wB")
            biasc = pers.tile([OUT_, 1], f32, name="biasc")
            TBb = pers.tile([OUT_, S_], wdt, name="TBb")        # termB[y,o]+bias
            termA = pers.tile([P, XB, OUT_], f32, name="termA")
            ident = pers.tile([P, P], f32, name="ident")
            identw = pers.tile([P, P], wdt, name="identw")
            if w1_bf16:
                in1Tx_b = pers.tile([P, KI, XW], wdt, name="in1Tx_b")

            # ---------------- prep: transposes + affine terms ----------------
            with tc.tile_pool(name="prep", bufs=2) as prep, \
                 tc.tile_pool(name="prep_ps", bufs=2, space="PSUM") as prep_ps:
                make_identity(nc, ident)
                nc.vector.tensor_copy(identw, ident)

                nc.sync.dma_start(wA, w2[0:IN_, :].rearrange("(a p) o -> p a o", p=P))
                nc.sync.dma_start(wB, w2[IN_:2 * IN_, :].rearrange("(a p) o -> p a o", p=P))
                with nc.allow_non_contiguous_dma(reason="112B one-time bias load"):
                    nc.sync.dma_start(biasc, w2[2 * IN_:2 * IN_ + 1, :].rearrange("a o -> o a"))

                def transpose_into(dst, src_dram, rows):
                    # src_dram: [rows, IN_] fp32 -> dst [P, KI, rows] (= src^T)
                    st = prep.tile([P, rows // P, IN_], f32, name="stage", tag="stage")
                    nc.sync.dma_start(st, src_dram[:, :].rearrange("(a p) i -> p a i", p=P))
                    for a in range(rows // P):
                        for ib in range(KI):
                            pt = prep_ps.tile([P, P], f32, name="pt", tag="pt")
                            nc.tensor.transpose(pt, st[:, a, ib * P:(ib + 1) * P], ident)
                            nc.vector.tensor_copy(dst[:, ib, a * P:(a + 1) * P], pt)

                transpose_into(in1Tx, in1x, XW)
                transpose_into(in1Tf, in1f, S_)
                transpose_into(in2T, in2f, S_)  # cast to wdt in the copy
                if w1_bf16:
                    nc.vector.tensor_copy(in1Tx_b, in1Tx)

                # TBb[o, y] = sum_j wB[j,o] * in1f[y,j] + bias[o]
                psTB = prep_ps.tile([OUT_, S_], f32, name="psTB", tag="psTB")
                for jb in range(KI):
                    nc.tensor.matmul(psTB, wB[:, jb, :], in1Tf[:, jb, :],
                                     start=(jb == 0), stop=(jb == KI - 1))
                nc.vector.tensor_scalar_add(TBb, psTB, biasc)

                # termA[x, o] = sum_i in1x[x,i] * wA[i,o]
                for xb in range(XB):
                    psA = prep_ps.tile([P, OUT_], f32, name="psA", tag="psA")
                    for ib in range(KI):
                        nc.tensor.matmul(psA, in1Tx[:, ib, xb * P:(xb + 1) * P],
                                         wA[:, ib, :],
                                         start=(ib == 0), stop=(ib == KI - 1))
                    nc.vector.tensor_copy(termA[:, xb, :], psA)

            # ---------------- main: o-chunked two-phase pipeline ----------------
            with tc.tile_pool(name="w1p", bufs=8) as w1p, \
                 tc.tile_pool(name="tempp", bufs=temp_bufs) as tempp, \
                 tc.tile_pool(name="outsb", bufs=3) as outsb, \
                 tc.tile_pool(name="ps1", bufs=4, space="PSUM") as ps1p, \
                 tc.tile_pool(name="ps2", bufs=4, space="PSUM") as ps2p:
                rhs1 = in1Tx_b if w1_bf16 else in1Tx
                for oc in [c for _ in range(repeat) for c in range(NCH)]:
                    # phase 1: temp[j, ol, x] for this o-chunk
                    temp = tempp.tile([P, KI, OC, XW], wdt, name="temp", tag="temp")
                    for og in range(OC // OB) if only_phase in (0, 1) else []:
                        w1t = []
                        for ib in range(KI):
                            t = w1p.tile([P, OB, IN_], wdt, name="w1t", tag="w1t")
                            nc.sync.dma_start(
                                t, w1[ib * P:(ib + 1) * P,
                                      oc * OC + og * OB:oc * OC + (og + 1) * OB, :])
                            w1t.append(t)
                        for bl in range(OB):
                            ol = og * OB + bl
                            if jb_pack:
                                # two j-block accumulation groups share one
                                # PSUM bank (disjoint column halves) so ONE
                                # DVE copy drains both -- halves the DVE op
                                # count (per-op DRAIN overhead dominates DVE)
                                for jp in range(KI // 2):
                                    ps1 = ps1p.tile([P, 2, XW], f32,
                                                    name="ps1", tag="ps1")
                                    for h in range(2):
                                        jb = 2 * jp + h
                                        for ib in range(KI):
                                            nc.tensor.matmul(
                                                ps1[:, h, :],
                                                w1t[ib][:, bl, jb * P:(jb + 1) * P],
                                                rhs1[:, ib, :],
                                                start=(ib == 0),
                                                stop=(ib == KI - 1))
                                    nc.vector.tensor_copy(
                                        temp[:, 2 * jp:2 * jp + 2, ol, :], ps1)
                            else:
                                for jb in range(KI):
                                    ps1 = ps1p.tile([P, XW], f32, name="ps1", tag="ps1")
                                    for ib in range(KI):
                                        nc.tensor.matmul(
                                            ps1, w1t[ib][:, bl, jb * P:(jb + 1) * P],
                                            rhs1[:, ib, :],
                                            start=(ib == 0), stop=(ib == KI - 1))
                                    nc.vector.tensor_copy(temp[:, jb, ol, :], ps1)
                    # phase 2: out[x, y] per o, + affine
                    for xb in range(XB) if only_phase in (0, 2) else []:
                        for g in range(OC // OG):
                            ot = outsb.tile([P, OG, S_], f32, name="ot", tag="ot")
                            for gl in range(OG):
                                ol = g * OG + gl
                                o = oc * OC + ol
                                ps2 = ps2p.tile([P, S_], f32, name="ps2", tag="ps2")
                                # selector matmul adds TBb[o, :] to every x row:
                                # lhsT[k, m] = ident[k, o] (free-broadcast), so
                                # out[m, n] += sum_k ident[k,o] * TBb[k,n] = TBb[o,n]
                                nc.tensor.matmul(
                                    ps2,
                                    identw[0:OUT_, o:o + 1].to_broadcast((OUT_, P)),
                                    TBb,
                                    start=True, stop=False)
                                for jb in range(KI):
                                    nc.tensor.matmul(
                                        ps2, temp[:, jb, ol, xb * P:(xb + 1) * P],
                                        in2T[:, jb, :],
                                        start=False, stop=(jb == KI - 1))
                                if act_drain:
                                    # PSUM drain + termA add on the idle ACT
                                    # engine: out = Identity(in*1 + bias)
                                    nc.scalar.activation(
                                        ot[:, gl, :], ps2,
                                        mybir.ActivationFunctionType.Identity,
                                        bias=termA[:, xb, o:o + 1])
                                else:
                                    nc.vector.tensor_scalar_add(
                                        ot[:, gl, :], ps2,
                                        termA[:, xb, o:o + 1])
                            nc.sync.dma_start(
                                outp[xb * P:(xb + 1) * P,
                                     oc * OC + g * OG:oc * OC + (g + 1) * OG, :],
                                ot)

    if split_waits:
        split_sync_waits(nc)
    return nc


_CACHE = {}


def _get_nc(**kw):
    key = tuple(sorted(kw.items()))
    if key not in _CACHE:
        _CACHE[key] = build_nc(**kw)
    return _CACHE[key]


W1_BF16 = True
TRACE = False
LAST_RESULT = None


def kernel(input1, input2, w1, w2, seq_len=None, **_ignored):
    global LAST_RESULT
    from concourse.bass_utils import run_bass_kernel_spmd
    import ml_dtypes

    input1 = np.asarray(input1, dtype=np.float32)
    input2 = np.asarray(input2, dtype=np.float32)
    w1 = np.asarray(w1, dtype=np.float32)
    w2 = np.asarray(w2, dtype=np.float32)

    nc = _get_nc(w1_bf16=W1_BF16)
    w1_dev = w1.astype(ml_dtypes.bfloat16) if W1_BF16 else w1

    XW = S // 2
    in_maps = []
    for c in range(N_CORES):
        b, xh = divmod(c, 2)
        x0 = xh * XW
        in_maps.append({
            "in1x": np.ascontiguousarray(input1[b, x0:x0 + XW, :]),
            "in1f": input1[b],
            "in2f": input2[b],
            "w1": w1_dev,
            "w2": w2,
        })
    res = run_bass_kernel_spmd(nc, in_maps, core_ids=list(range(N_CORES)),
                               trace=TRACE)
    LAST_RESULT = res

    full = np.empty((B, S, S, OUT), dtype=np.float32)
    for c in range(N_CORES):
        b, xh = divmod(c, 2)
        x0 = xh * XW
        # device layout [x, o, y] -> [x, y, o]
        full[b, x0:x0 + XW] = res.results[c]["outp"].transpose(0, 2, 1)
    return full



# revision 5
# speedup vs baseline: 1.1104x; 1.1104x over previous
"""Biaffine kernel for Trainium2, 8-core SPMD — OUT-sharded (v2).

logits[b,x,y,o] = sum_ij in1[b,x,i] * w1[i,o,j] * in2[b,y,j]
               + termA[b,x,o] + termB[b,y,o] + bias[o]

Sharding: core c owns the o-slice [14c, 14c+14) and computes ALL (b, x, y)
for it.  w1's o-slice (7.3 MB bf16) is SBUF-RESIDENT — no weight streaming
during the main loop (the previous x-sharded kernel streamed the full
58.7 MB w1 through every core, putting DMA on the tensor-engine ridge:
~360us DMA vs ~380us PE per core; this design needs only ~35 MB total
DMA per core, all overlappable).

Per (b, o):
  phase 1: temp[j, x] = sum_i w1[i,o,j] * in1[b,x,i]
           stationary = w1 128x128 tile (resident), moving = in1T [128, 512]
           -> 16 MMs of N=512 (ldweights ~107ns hides under the 213ns
           moving stream via the PE pull-ahead window), ACT drains
           PSUM->SBUF bf16.
  phase 2: out[x, y] = sum_jb temp[jb, x-tile]^T @ in2T[jb, y]
           -> 16 MMs of N=512; one fused DVE op drains PSUM AND applies
           the ENTIRE affine: out = (PSUM + termA[x,o]) + TBA[o]
           (scalar_tensor_tensor with per-partition scalar termA).
           TBA[o] = (termB[b,:,o]+bias[o]) row broadcast across partitions,
           built by ONE selector matmul per (b,o) — spread one per
           o-iteration so its ACT copies never serialize the PE stream —
           instead of one per output tile like the old kernel.
temp double-buffered: phase 1 of o+1 overlaps phase 2 of o.
Inputs are passed pre-cast to bf16 (everything consumes bf16; halves
input DMA) and in1T/in2T are built by xbar DMA transposes (k-major:
dst[p,k,x] = src[x, k*128+p], device-verified) with no PE/DVE involvement.
Output is written bf16 ([b, o, x, y]; +0.0008 rel err vs the 2e-2 gate)
and upcast/transposed to [b, x, y, o] fp32 on the host.

Measured (sim = concourse TimelineSim; HW = repeat-delta wall clock on the
axon-tunneled device, donation-based bench.py):
  old x-sharded kernel: sim 474us, HW main-loop ~557us/rep (harness: 739us)
  this kernel:          sim 441us, HW main-loop ~449us/rep
PE occupancy in sim is 90%+; PSUM banks 3(ph1)+3(ph2)+1(selector)+1(termB).
"""

import numpy as np

B, S, IN, OUT = 4, 512, 512, 112
N_CORES = 8
P = 128
OC = OUT // N_CORES  # 14 o's per core


def split_sync_waits(nc, max_waits=1):
    """Hoist overflow semaphore waits onto NoOps (walrus rejects
    instructions with too many sync waits)."""
    import concourse.mybir as mybir

    n_split = 0
    for f in nc.m.functions:
        for bb in f.blocks:
            new_insts = []
            for inst in bb.instructions:
                si = inst.sync_info
                if si is not None and si.on_wait and len(si.on_wait) > max_waits:
                    waits = list(si.on_wait)
                    overflow, keep = waits[:-max_waits], waits[-max_waits:]
                    for k in range(0, len(overflow), max_waits):
                        chunk = overflow[k:k + max_waits]
                        nop = mybir.InstNoOp(
                            name=f"{inst.name}_wsplit{k}",
                            opcode="NoOp",
                            engine=inst.engine,
                            sync_info=mybir.SyncInfo(on_wait=chunk, on_update=[]),
                        )
                        new_insts.append(nop)
                        n_split += 1
                    si.on_wait = keep
                new_insts.append(inst)
            bb.instructions[:] = new_insts
    return n_split


def build_nc(S_=S, IN_=IN, OC_=OC, ps1_bufs=3, ps2_bufs=3, psb_bufs=1,
             temp_bufs=2, out_bufs=4, split_waits=True, repeat=1, only_phase=0,
             out_bf16=True, dma_tr=True):
    import concourse.bass as bass
    import concourse.mybir as mybir
    import concourse.tile as tile
    from concourse.masks import make_identity

    f32 = mybir.dt.float32
    bf16 = mybir.dt.bfloat16
    odt = bf16 if out_bf16 else f32

    KI = IN_ // P   # 128-blocks of the i/j contraction dims (4)
    XB = S_ // P    # x 128-blocks (4)

    nc = bass.Bass()
    in1 = nc.dram_tensor("in1", [B, S_, IN_], bf16, kind="ExternalInput")
    in2 = nc.dram_tensor("in2", [B, S_, IN_], bf16, kind="ExternalInput")
    w1s = nc.dram_tensor("w1s", [IN_, OC_, IN_], bf16, kind="ExternalInput")
    w2s = nc.dram_tensor("w2s", [2 * IN_ + 1, OC_], f32, kind="ExternalInput")
    outp = nc.dram_tensor("outp", [B, OC_, S_, S_], odt, kind="ExternalOutput")

    with tile.TileContext(nc) as tc:
        with tc.tile_pool(name="persist", bufs=1) as pers:
            in1T = pers.tile([P, B, KI, S_], bf16, name="in1T")
            in2T = pers.tile([P, B, KI, S_], bf16, name="in2T")
            w1sb = pers.tile([P, KI, OC_, IN_], bf16, name="w1sb")
            wAsb = pers.tile([P, KI, OC_], bf16, name="wAsb")
            wBsb = pers.tile([P, KI, OC_], bf16, name="wBsb")
            biasc = pers.tile([OC_, 1], f32, name="biasc")
            termA = pers.tile([P, B, XB, OC_], f32, name="termA")
            TBA = pers.tile([P, 2, OC_, S_], bf16, name="TBA")  # per b-parity
            ident = pers.tile([P, P], f32, name="ident")
            identw = pers.tile([P, P], bf16, name="identw")

            # ---------------- prep: loads + transposes + termA ----------------
            with tc.tile_pool(name="prep", bufs=2) as prep, \
                 tc.tile_pool(name="prep_ps", bufs=2, space="PSUM") as prep_ps:
                make_identity(nc, ident)
                nc.vector.tensor_copy(identw, ident)

                def transpose_into(dst, src_dram):
                    # src_dram: [S_, IN_] bf16 -> dst [P, KI, S_] bf16 (= src^T)
                    if dma_tr:
                        # xbar DMA transpose straight from DRAM: no PE/DVE
                        # involvement.  Layout is k-major (dst[p,k,x] =
                        # src[x, k*128+p]) — verified by device probe.
                        nc.sync.dma_start(dst[:, :, :], src_dram[:, :],
                                          transpose=True)
                        return
                    # staged in XB chunks so the first transpose starts after
                    # ~1/4 of the load; bf16 PE transposes run 1 cycle/row and
                    # the bf16 PSUM->SBUF copies get the 2x DVE mode
                    st = prep.tile([P, XB, IN_], bf16, name="stage", tag="stage")
                    for a in range(XB):
                        nc.sync.dma_start(
                            st[:, a, :],
                            src_dram[a * P:(a + 1) * P, :].rearrange(
                                "(a p) i -> p (a i)", p=P))
                    for a in range(XB):
                        for ib in range(KI):
                            pt = prep_ps.tile([P, P], bf16, name="pt", tag="pt")
                            nc.tensor.transpose(pt, st[:, a, ib * P:(ib + 1) * P],
                                                identw)
                            nc.vector.tensor_copy(dst[:, ib, a * P:(a + 1) * P], pt)

                # input stages go FIRST so the transpose pipeline starts
                # immediately; the (big) resident-w1 load queues behind them
                # and finishes well before the first main-loop matmul needs it.
                transpose_into(in1T[:, 0], in1[0])
                wABf = prep.tile([P, 2, KI, OC_], f32, name="wABf", tag="wABf")
                nc.sync.dma_start(
                    wABf[:, 0], w2s[0:IN_, :].rearrange("(a p) o -> p a o", p=P))
                nc.sync.dma_start(
                    wABf[:, 1], w2s[IN_:2 * IN_, :].rearrange("(a p) o -> p a o", p=P))
                nc.vector.tensor_copy(wAsb, wABf[:, 0])
                nc.vector.tensor_copy(wBsb, wABf[:, 1])
                with nc.allow_non_contiguous_dma(reason="56B one-time bias load"):
                    nc.sync.dma_start(
                        biasc, w2s[2 * IN_:2 * IN_ + 1, :].rearrange("a o -> o a"))
                transpose_into(in2T[:, 0], in2[0])
                for b in range(1, B):
                    transpose_into(in1T[:, b], in1[b])
                    transpose_into(in2T[:, b], in2[b])
                for ib in range(KI):
                    nc.sync.dma_start(w1sb[:, ib, :, :],
                                      w1s[ib * P:(ib + 1) * P, :, :])

                # termA[x, o] = sum_i in1[b,x,i] * wA[i,o]
                for b in range(B):
                    for xb in range(XB):
                        psA = prep_ps.tile([P, OC_], f32, name="psA", tag="psA")
                        for ib in range(KI):
                            nc.tensor.matmul(
                                psA, in1T[:, b, ib, xb * P:(xb + 1) * P],
                                wAsb[:, ib, :],
                                start=(ib == 0), stop=(ib == KI - 1))
                        nc.vector.tensor_copy(termA[:, b, xb, :], psA)

            # ---------------- main: per (b, o) two-phase pipeline ----------------
            with tc.tile_pool(name="tempp", bufs=temp_bufs) as tempp, \
                 tc.tile_pool(name="tbap", bufs=2) as tbap, \
                 tc.tile_pool(name="outsb", bufs=out_bufs) as outsb, \
                 tc.tile_pool(name="ps1", bufs=ps1_bufs, space="PSUM") as ps1p, \
                 tc.tile_pool(name="ps2", bufs=ps2_bufs, space="PSUM") as ps2p, \
                 tc.tile_pool(name="pstb", bufs=1, space="PSUM") as pstbap, \
                 tc.tile_pool(name="psb", bufs=psb_bufs, space="PSUM") as psbp:
                tbbs = {}

                def tba_termbt(b, slot):
                    # termB[b, :, o] + bias[o] on o-partitions [OC_, S_]
                    psTB = pstbap.tile([OC_, S_], f32, name="psTB", tag="psTB")
                    for jb in range(KI):
                        nc.tensor.matmul(psTB, wBsb[:, jb, :], in1T[:, b, jb, :],
                                         start=(jb == 0), stop=(jb == KI - 1))
                    tbb = tbap.tile([OC_, S_], bf16, name="tbb", tag="tbb")
                    nc.vector.tensor_scalar_add(tbb, psTB, biasc)
                    tbbs[slot] = tbb

                def tba_selector(slot, ol):
                    # TBA[slot, ol, :] = tbb[ol, :] broadcast over partitions
                    psb = psbp.tile([P, S_], f32, name="psb", tag="psb")
                    nc.tensor.matmul(
                        psb, identw[0:OC_, ol:ol + 1].to_broadcast((OC_, P)),
                        tbbs[slot], start=True, stop=True)
                    nc.scalar.activation(TBA[:, slot, ol, :], psb,
                                         mybir.ActivationFunctionType.Copy)

                bseq = [bb for _ in range(repeat) for bb in range(B)]
                if bseq:
                    # b0's TBA is built inside its own o-loop: selector(ol)
                    # lands at iteration ol//2, always before phase2(ol).
                    tba_termbt(bseq[0], 0)
                    for ol in range(2):
                        tba_selector(0, ol)
                for bi, b in enumerate(bseq):
                    par = bi % 2
                    for ol in range(OC_):
                        # spread TBA work: finish this b's own selectors
                        # (first iterations), then build next b's TBA one
                        # selector per iteration — ACT copies overlap main
                        # matmuls instead of serializing the PE stream.
                        if bi == 0 and ol < OC_ // 2:
                            for x in (2 * ol + 2, 2 * ol + 3):
                                if x < OC_:
                                    tba_selector(0, x)
                        if bi + 1 < len(bseq):
                            nslot = (bi + 1) % 2
                            if ol == 0:
                                tba_termbt(bseq[bi + 1], nslot)
                            else:
                                tba_selector(nslot, ol - 1)
                                if ol == OC_ - 1:
                                    tba_selector(nslot, ol)
                        # phase 1: temp[j, x] for this (b, o)
                        temp = tempp.tile([P, KI, S_], bf16, name="temp", tag="temp")
                        for jb in range(KI) if only_phase in (0, 1) else []:
                            ps1 = ps1p.tile([P, S_], f32, name="ps1", tag="ps1")
                            for ib in range(KI):
                                nc.tensor.matmul(
                                    ps1, w1sb[:, ib, ol, jb * P:(jb + 1) * P],
                                    in1T[:, b, ib, :],
                                    start=(ib == 0), stop=(ib == KI - 1))
                            nc.scalar.activation(
                                temp[:, jb, :], ps1,
                                mybir.ActivationFunctionType.Copy)
                        # phase 2: out[x, y] + affine
                        for xb in range(XB) if only_phase in (0, 2) else []:
                            ps2 = ps2p.tile([P, S_], f32, name="ps2", tag="ps2")
                            for jb in range(KI):
                                nc.tensor.matmul(
                                    ps2, temp[:, jb, xb * P:(xb + 1) * P],
                                    in2T[:, b, jb, :],
                                    start=(jb == 0), stop=(jb == KI - 1))
                            ot = outsb.tile([P, S_], odt, name="ot", tag="ot")
                            nc.vector.scalar_tensor_tensor(
                                ot, ps2, termA[:, b, xb, ol:ol + 1],
                                TBA[:, par, ol, :],
                                mybir.AluOpType.add, mybir.AluOpType.add)
                            nc.sync.dma_start(
                                outp[b, ol, xb * P:(xb + 1) * P, :], ot)

    if split_waits:
        split_sync_waits(nc)
    return nc


_CACHE = {}


def _get_nc(**kw):
    key = tuple(sorted(kw.items()))
    if key not in _CACHE:
        _CACHE[key] = build_nc(**kw)
    return _CACHE[key]


TRACE = False
OUT_BF16 = True
LAST_RESULT = None


def kernel(input1, input2, w1, w2, seq_len=None, **_ignored):
    global LAST_RESULT
    from concourse.bass_utils import run_bass_kernel_spmd
    import ml_dtypes

    input1 = np.asarray(input1, dtype=np.float32)
    input2 = np.asarray(input2, dtype=np.float32)
    w1 = np.asarray(w1, dtype=np.float32)
    w2 = np.asarray(w2, dtype=np.float32)

    nc = _get_nc(out_bf16=OUT_BF16)
    w1b = w1.astype(ml_dtypes.bfloat16)

    in1b = input1.astype(ml_dtypes.bfloat16)
    in2b = input2.astype(ml_dtypes.bfloat16)
    in_maps = []
    for c in range(N_CORES):
        o0 = c * OC
        in_maps.append({
            "in1": in1b,
            "in2": in2b,
            "w1s": np.ascontiguousarray(w1b[:, o0:o0 + OC, :]),
            "w2s": np.ascontiguousarray(w2[:, o0:o0 + OC]),
        })
    res = run_bass_kernel_spmd(nc, in_maps, core_ids=list(range(N_CORES)),
                               trace=TRACE)
    LAST_RESULT = res

    full = np.empty((B, S, S, OUT), dtype=np.float32)
    for c in range(N_CORES):
        o0 = c * OC
        # device layout [b, o, x, y] -> [b, x, y, o]
        full[:, :, :, o0:o0 + OC] = np.asarray(
            res.results[c]["outp"], dtype=np.float32).transpose(0, 2, 3, 1)
    return full


# revision 9
# speedup vs baseline: 1.1123x; 1.0017x over previous
"""Biaffine kernel for Trainium2, 8-core SPMD — OUT-sharded (v2).

logits[b,x,y,o] = sum_ij in1[b,x,i] * w1[i,o,j] * in2[b,y,j]
               + termA[b,x,o] + termB[b,y,o] + bias[o]

Sharding: core c owns the o-slice [14c, 14c+14) and computes ALL (b, x, y)
for it.  w1's o-slice (7.3 MB bf16) is SBUF-RESIDENT — no weight streaming
during the main loop (the previous x-sharded kernel streamed the full
58.7 MB w1 through every core, putting DMA on the tensor-engine ridge:
~360us DMA vs ~380us PE per core; this design needs only ~35 MB total
DMA per core, all overlappable).

Per (b, o):
  phase 1: temp[j, x] = sum_i w1[i,o,j] * in1[b,x,i]
           stationary = w1 128x128 tile (resident), moving = in1T [128, 512]
           -> 16 MMs of N=512 (ldweights ~107ns hides under the 213ns
           moving stream via the PE pull-ahead window), ACT drains
           PSUM->SBUF bf16.
  phase 2: out[x, y] = sum_jb temp[jb, x-tile]^T @ in2T[jb, y]
           -> 16 MMs of N=512; one fused DVE op drains PSUM AND applies
           the ENTIRE affine: out = (PSUM + termA[x,o]) + TBA[o]
           (scalar_tensor_tensor with per-partition scalar termA).
           TBA[o] = (termB[b,:,o]+bias[o]) row broadcast across partitions,
           built by ONE selector matmul per (b,o) — spread one per
           o-iteration so its ACT copies never serialize the PE stream —
           instead of one per output tile like the old kernel.
temp double-buffered: phase 1 of o+1 overlaps phase 2 of o.
Inputs are passed pre-cast to bf16 (everything consumes bf16; halves
input DMA) and in1T/in2T are built by xbar DMA transposes (k-major:
dst[p,k,x] = src[x, k*128+p], device-verified) with no PE/DVE involvement.
Output is written bf16 ([b, o, x, y]; +0.0008 rel err vs the 2e-2 gate)
and upcast/transposed to [b, x, y, o] fp32 on the host.

Measured (sim = concourse TimelineSim; HW = repeat-delta wall clock on the
axon-tunneled device, donation-based bench.py):
  old x-sharded kernel: sim 474us, HW main-loop ~557us/rep (harness: 739us)
  this kernel:          sim 441us, HW main-loop ~449us/rep
PE occupancy in sim is 90%+; PSUM banks 3(ph1)+3(ph2)+1(selector)+1(termB).
"""

import numpy as np

B, S, IN, OUT = 4, 512, 512, 112
N_CORES = 8
P = 128
OC = OUT // N_CORES  # 14 o's per core


def split_sync_waits(nc, max_waits=1):
    """Hoist overflow semaphore waits onto NoOps (walrus rejects
    instructions with too many sync waits)."""
    import concourse.mybir as mybir

    n_split = 0
    for f in nc.m.functions:
        for bb in f.blocks:
            new_insts = []
            for inst in bb.instructions:
                si = inst.sync_info
                if si is not None and si.on_wait and len(si.on_wait) > max_waits:
                    waits = list(si.on_wait)
                    overflow, keep = waits[:-max_waits], waits[-max_waits:]
                    for k in range(0, len(overflow), max_waits):
                        chunk = overflow[k:k + max_waits]
                        nop = mybir.InstNoOp(
                            name=f"{inst.name}_wsplit{k}",
                            opcode="NoOp",
                            engine=inst.engine,
                            sync_info=mybir.SyncInfo(on_wait=chunk, on_update=[]),
                        )
                        new_insts.append(nop)
                        n_split += 1
                    si.on_wait = keep
                new_insts.append(inst)
            bb.instructions[:] = new_insts
    return n_split


def build_nc(S_=S, IN_=IN, OC_=OC, ps1_bufs=4, ps2_bufs=2, psb_bufs=1,
             temp_bufs=2, out_bufs=4, split_waits=True, repeat=1, only_phase=0,
             out_bf16=True, dma_tr=True):
    import concourse.bass as bass
    import concourse.mybir as mybir
    import concourse.tile as tile
    from concourse.masks import make_identity

    f32 = mybir.dt.float32
    bf16 = mybir.dt.bfloat16
    odt = bf16 if out_bf16 else f32

    KI = IN_ // P   # 128-blocks of the i/j contraction dims (4)
    XB = S_ // P    # x 128-blocks (4)

    nc = bass.Bass()
    in1 = nc.dram_tensor("in1", [B, S_, IN_], bf16, kind="ExternalInput")
    in2 = nc.dram_tensor("in2", [B, S_, IN_], bf16, kind="ExternalInput")
    w1s = nc.dram_tensor("w1s", [IN_, OC_, IN_], bf16, kind="ExternalInput")
    w2s = nc.dram_tensor("w2s", [2 * IN_ + 1, OC_], f32, kind="ExternalInput")
    outp = nc.dram_tensor("outp", [B, OC_, S_, S_], odt, kind="ExternalOutput")

    with tile.TileContext(nc) as tc:
        with tc.tile_pool(name="persist", bufs=1) as pers:
            in1T = pers.tile([P, B, KI, S_], bf16, name="in1T")
            in2T = pers.tile([P, B, KI, S_], bf16, name="in2T")
            w1sb = pers.tile([P, KI, OC_, IN_], bf16, name="w1sb")
            wAsb = pers.tile([P, KI, OC_], bf16, name="wAsb")
            wBsb = pers.tile([P, KI, OC_], bf16, name="wBsb")
            biasc = pers.tile([OC_, 1], f32, name="biasc")
            termA = pers.tile([P, B, XB, OC_], f32, name="termA")
            tbb_all = pers.tile([OC_, B, S_], bf16, name="tbb_all")
            TBA = pers.tile([P, 2, OC_, S_], bf16, name="TBA")  # per b-parity
            ident = pers.tile([P, P], f32, name="ident")
            identw = pers.tile([P, P], bf16, name="identw")

            # ---------------- prep: loads + transposes + termA ----------------
            with tc.tile_pool(name="prep", bufs=2) as prep, \
                 tc.tile_pool(name="prep_ps", bufs=2, space="PSUM") as prep_ps:
                make_identity(nc, ident)
                nc.vector.tensor_copy(identw, ident)

                def transpose_into(dst, src_dram):
                    # src_dram: [S_, IN_] bf16 -> dst [P, KI, S_] bf16 (= src^T)
                    if dma_tr:
                        # xbar DMA transpose straight from DRAM: no PE/DVE
                        # involvement.  Layout is k-major (dst[p,k,x] =
                        # src[x, k*128+p]) — verified by device probe.
                        nc.sync.dma_start(dst[:, :, :], src_dram[:, :],
                                          transpose=True)
                        return
                    # staged in XB chunks so the first transpose starts after
                    # ~1/4 of the load; bf16 PE transposes run 1 cycle/row and
                    # the bf16 PSUM->SBUF copies get the 2x DVE mode
                    st = prep.tile([P, XB, IN_], bf16, name="stage", tag="stage")
                    for a in range(XB):
                        nc.sync.dma_start(
                            st[:, a, :],
                            src_dram[a * P:(a + 1) * P, :].rearrange(
                                "(a p) i -> p (a i)", p=P))
                    for a in range(XB):
                        for ib in range(KI):
                            pt = prep_ps.tile([P, P], bf16, name="pt", tag="pt")
                            nc.tensor.transpose(pt, st[:, a, ib * P:(ib + 1) * P],
                                                identw)
                            nc.vector.tensor_copy(dst[:, ib, a * P:(a + 1) * P], pt)

                # input stages go FIRST so the transpose pipeline starts
                # immediately; the (big) resident-w1 load queues behind them
                # and finishes well before the first main-loop matmul needs it.
                transpose_into(in1T[:, 0], in1[0])
                wABf = prep.tile([P, 2, KI, OC_], f32, name="wABf", tag="wABf")
                nc.sync.dma_start(
                    wABf[:, 0], w2s[0:IN_, :].rearrange("(a p) o -> p a o", p=P))
                nc.sync.dma_start(
                    wABf[:, 1], w2s[IN_:2 * IN_, :].rearrange("(a p) o -> p a o", p=P))
                nc.vector.tensor_copy(wAsb, wABf[:, 0])
                nc.vector.tensor_copy(wBsb, wABf[:, 1])
                with nc.allow_non_contiguous_dma(reason="56B one-time bias load"):
                    nc.sync.dma_start(
                        biasc, w2s[2 * IN_:2 * IN_ + 1, :].rearrange("a o -> o a"))
                transpose_into(in2T[:, 0], in2[0])
                for b in range(1, B):
                    transpose_into(in1T[:, b], in1[b])
                    transpose_into(in2T[:, b], in2[b])
                # w1 load last: splitting it into an urgent first o-chunk or
                # hoisting it ahead of the b1-3 input loads measured WORSE
                # (444.8us vs 440.0us) — the main loop's early iterations do
                # not actually wait on it.
                for ib in range(KI):
                    nc.sync.dma_start(w1sb[:, ib, :, :],
                                      w1s[ib * P:(ib + 1) * P, :, :])

                # termA[x, o] = sum_i in1[b,x,i] * wA[i,o]
                for b in range(B):
                    for xb in range(XB):
                        psA = prep_ps.tile([P, OC_], f32, name="psA", tag="psA")
                        for ib in range(KI):
                            nc.tensor.matmul(
                                psA, in1T[:, b, ib, xb * P:(xb + 1) * P],
                                wAsb[:, ib, :],
                                start=(ib == 0), stop=(ib == KI - 1))
                        nc.vector.tensor_copy(termA[:, b, xb, :], psA)

                # tbb_all[o, b, y] = termB[b, y, o] + bias[o]
                for b in range(B):
                    psTB = prep_ps.tile([OC_, S_], f32, name="psTB", tag="psTB")
                    for jb in range(KI):
                        nc.tensor.matmul(psTB, wBsb[:, jb, :], in1T[:, b, jb, :],
                                         start=(jb == 0), stop=(jb == KI - 1))
                    nc.vector.tensor_scalar_add(tbb_all[:, b, :], psTB, biasc)

            # ---------------- main: per (b, o) two-phase pipeline ----------------
            with tc.tile_pool(name="tempp", bufs=temp_bufs) as tempp, \
                 tc.tile_pool(name="outsb", bufs=out_bufs) as outsb, \
                 tc.tile_pool(name="ps1", bufs=ps1_bufs, space="PSUM") as ps1p, \
                 tc.tile_pool(name="ps2", bufs=ps2_bufs, space="PSUM") as ps2p, \
                 tc.tile_pool(name="psb", bufs=psb_bufs, space="PSUM") as psbp:
                def tba_selector(b, slot, ol):
                    # TBA[slot, ol, :] = tbb_all[ol, b, :] bcast over partitions
                    psb = psbp.tile([P, S_], f32, name="psb", tag="psb")
                    nc.tensor.matmul(
                        psb, identw[0:OC_, ol:ol + 1].to_broadcast((OC_, P)),
                        tbb_all[:, b, :], start=True, stop=True)
                    nc.scalar.activation(TBA[:, slot, ol, :], psb,
                                         mybir.ActivationFunctionType.Copy)

                bseq = [bb for _ in range(repeat) for bb in range(B)]
                for bi, b in enumerate(bseq):
                    par = bi % 2
                    for ol in range(OC_):
                        # one selector per iteration, just-in-time (termB rows
                        # all precomputed in prep; psb double-buffered so even
                        # scheduler-bunched selectors don't stall the PE)
                        tba_selector(b, par, ol)
                        # phase 1: temp[j, x] for this (b, o)
                        # (pairing jb's into 2-bank psum tiles with one wide
                        # ACT drain measured neutral-to-worse; keep singles)
                        temp = tempp.tile([P, KI, S_], bf16, name="temp", tag="temp")
                        for jb in range(KI) if only_phase in (0, 1) else []:
                            ps1 = ps1p.tile([P, S_], f32, name="ps1", tag="ps1")
                            for ib in range(KI):
                                nc.tensor.matmul(
                                    ps1, w1sb[:, ib, ol, jb * P:(jb + 1) * P],
                                    in1T[:, b, ib, :],
                                    start=(ib == 0), stop=(ib == KI - 1))
                            nc.scalar.activation(
                                temp[:, jb, :], ps1,
                                mybir.ActivationFunctionType.Copy)
                        # phase 2: out[x, y] + affine
                        for xb in range(XB) if only_phase in (0, 2) else []:
                            ps2 = ps2p.tile([P, S_], f32, name="ps2", tag="ps2")
                            for jb in range(KI):
                                nc.tensor.matmul(
                                    ps2, temp[:, jb, xb * P:(xb + 1) * P],
                                    in2T[:, b, jb, :],
                                    start=(jb == 0), stop=(jb == KI - 1))
                            ot = outsb.tile([P, S_], odt, name="ot", tag="ot")
                            nc.vector.scalar_tensor_tensor(
                                ot, ps2, termA[:, b, xb, ol:ol + 1],
                                TBA[:, par, ol, :],
                                mybir.AluOpType.add, mybir.AluOpType.add)
                            nc.sync.dma_start(
                                outp[b, ol, xb * P:(xb + 1) * P, :], ot)

    if split_waits:
        # max_waits=1 is a HARD walrus limit: 2-deep waits fail codegen
        # (setupSyncWait, CoreV2GenImpl.cpp:176) — verified 2026-08.
        split_sync_waits(nc)
    return nc


_CACHE = {}


def _get_nc(**kw):
    key = tuple(sorted(kw.items()))
    if key not in _CACHE:
        _CACHE[key] = build_nc(**kw)
    return _CACHE[key]


TRACE = False
OUT_BF16 = True
LAST_RESULT = None


def kernel(input1, input2, w1, w2, seq_len=None, **_ignored):
    global LAST_RESULT
    from concourse.bass_utils import run_bass_kernel_spmd
    import ml_dtypes

    input1 = np.asarray(input1, dtype=np.float32)
    input2 = np.asarray(input2, dtype=np.float32)
    w1 = np.asarray(w1, dtype=np.float32)
    w2 = np.asarray(w2, dtype=np.float32)

    nc = _get_nc(out_bf16=OUT_BF16)
    w1b = w1.astype(ml_dtypes.bfloat16)

    in1b = input1.astype(ml_dtypes.bfloat16)
    in2b = input2.astype(ml_dtypes.bfloat16)
    in_maps = []
    for c in range(N_CORES):
        o0 = c * OC
        in_maps.append({
            "in1": in1b,
            "in2": in2b,
            "w1s": np.ascontiguousarray(w1b[:, o0:o0 + OC, :]),
            "w2s": np.ascontiguousarray(w2[:, o0:o0 + OC]),
        })
    res = run_bass_kernel_spmd(nc, in_maps, core_ids=list(range(N_CORES)),
                               trace=TRACE)
    LAST_RESULT = res

    full = np.empty((B, S, S, OUT), dtype=np.float32)
    for c in range(N_CORES):
        o0 = c * OC
        # device layout [b, o, x, y] -> [b, x, y, o]
        full[:, :, :, o0:o0 + OC] = np.asarray(
            res.results[c]["outp"], dtype=np.float32).transpose(0, 2, 3, 1)
    return full
